# revision 17
# baseline (speedup 1.0000x reference)
"""Trainium2 Bass kernel for nn_Attention_34325378629934 (XCA-style channel attention).

Sharding: 8 cores = 4 batches x 2 spatial halves (128 rows each).
Per core, pass 1 (per 16-row chunk):
  1x1 qkv conv as PE matmul (bias via ones-channel, K=193), 16 rows/chunk
    with a 2-row DVE carry of the depthwise halo from the previous chunk
  depthwise 3x3 split across engines per (tile, chunk):
    dve:   tensor_scalar (4x) + tensor_tensor (2x) per tap
    pe:    9 PSUM-accumulated matmuls with diagonal lhsT (shifts via AP offsets)
    actgp: ACT per-partition scale-mul + GpSimd tensor_tensor add
  tile 4 (64 ch) packs two chunks onto 128 partitions (weights duplicated)
  q/k head Gram via PE transpose + PSUM-accumulated PE matmuls
  pairwise AllReduce of Gram stats between the 2 cores of each batch
Pass 2: y = (P @ blockdiag(attn)) @ v with PA computed on device; proj bias via
  an appended ones-row in v; y DMA'd straight from PSUM (fp32).
"""
import sys
from contextlib import ExitStack

sys.path.insert(0, "/opt/trn_rl_repo")

import numpy as np
import ml_dtypes

import concourse.bass as bass
import concourse.mybir as mybir
import concourse.tile as tile
from concourse import bacc
from concourse.bass_utils import run_bass_kernel_spmd
from concourse.masks import make_identity

BF16 = ml_dtypes.bfloat16
f32 = mybir.dt.float32
bf16 = mybir.dt.bfloat16

N_CORES = 8
B, C, H, W = 4, 192, 256, 256
C3 = 3 * C
HEADS, HC = 8, 24
RH = 128                 # rows per core
S = RH * W               # 32768
R = 16                   # rows per chunk
CHUNKS = RH // R         # 8
KAUG = C + 1             # 193
EPS = 1e-12
Ident = mybir.ActivationFunctionType.Identity

TAPS = [(dy, dx) for dy in range(3) for dx in range(3)]  # center = index 4

# dw engine assignment per (tile, chunk); tile 4 handled per pair ("pe")
ASSIGN = {
    0: ["dve"] * 8,
    1: ["dve"] * 6 + ["actgp", "pe"],
    2: ["pe"] * 8,
    3: ["actgp"] * 6 + ["dve"] * 2,
}
T4_MODE = "pe"
QKT_SPLIT = True   # alternate qkT copies between ACT and DVE

_COMPILED = {}


def _build_nc(cfg=None):
    cfg = cfg or {}
    assign = cfg.get("assign", ASSIGN)
    t4_mode = cfg.get("t4", T4_MODE)
    nc = bacc.Bacc()

    x_ext = nc.declare_dram_parameter("x", [KAUG, R + 2, CHUNKS, W], bf16, isOutput=False)
    wq_ext = nc.declare_dram_parameter("wq", [5, KAUG, 128], bf16, isOutput=False)
    dww_ext = nc.declare_dram_parameter("dww", [5, 128, 9], f32, isOutput=False)
    dwb_ext = nc.declare_dram_parameter("dwb", [5, 128, 1], f32, isOutput=False)
    wpT_ext = nc.declare_dram_parameter("wpT", [2, 128, 192], bf16, isOutput=False)
    pb_ext = nc.declare_dram_parameter("pb", [1, 192], bf16, isOutput=False)
    scl_ext = nc.declare_dram_parameter("scl", [2, 96, 1], f32, isOutput=False)
    y_ext = nc.declare_dram_parameter("y", [C, S], bf16, isOutput=True)

    with tile.TileContext(nc) as tc, ExitStack() as ctx:
        consts = ctx.enter_context(tc.tile_pool(name="consts", bufs=1))
        xpool = ctx.enter_context(tc.tile_pool(name="xpool", bufs=2))
        inbp = [ctx.enter_context(tc.tile_pool(name=f"inb{t}", bufs=2))
                for t in range(5)]
        accp = [ctx.enter_context(tc.tile_pool(name=f"acc{t}", bufs=2))
                for t in range(5)]
        tmpd = ctx.enter_context(tc.tile_pool(name="tmpd", bufs=2))
        tmpg = ctx.enter_context(tc.tile_pool(name="tmpg", bufs=2))
        qkt = ctx.enter_context(tc.tile_pool(name="qkt", bufs=1))
        smallp = ctx.enter_context(tc.tile_pool(name="smallp", bufs=1))
        iop = ctx.enter_context(tc.tile_pool(name="iop", bufs=2))
        dram = ctx.enter_context(tc.tile_pool(name="dram", bufs=1, space="DRAM"))
        ps_row = ctx.enter_context(tc.tile_pool(name="ps_row", bufs=4, space="PSUM"))
        ps_tr = ctx.enter_context(tc.tile_pool(name="ps_tr", bufs=2, space="PSUM"))
        ps_gram = ctx.enter_context(tc.tile_pool(name="ps_gram", bufs=1, space="PSUM"))

        # ---------------- constants ----------------
        ident = consts.tile([128, 128], bf16)
        make_identity(nc, ident)
        wq_sb = []
        for t in range(5):
            k0 = consts.tile([128, 128], bf16, tag=f"wq{t}a")
            k1 = consts.tile([65, 128], bf16, tag=f"wq{t}b")
            nc.sync.dma_start(out=k0[:], in_=wq_ext[t, 0:128, :])
            nc.sync.dma_start(out=k1[:], in_=wq_ext[t, 128:KAUG, :])
            wq_sb.append((k0, k1))
        dww_sb, dwb_sb = [], []
        for t in range(5):
            dwt = consts.tile([128, 9], f32, tag=f"dww{t}")
            nc.sync.dma_start(out=dwt[:], in_=dww_ext[t])
            dww_sb.append(dwt)
            dbt = consts.tile([128, 1], f32, tag=f"dwb{t}")
            nc.sync.dma_start(out=dbt[:], in_=dwb_ext[t])
            dwb_sb.append(dbt)
        # diagonal dw-weight matrices for the PE path (only for PE-assigned tiles)
        pe_tiles = {t for t in range(4) if "pe" in assign[t]}
        if t4_mode == "pe":
            pe_tiles.add(4)
        diag_sb = {}
        for t in sorted(pe_tiles):
            for tap in range(9):
                d = consts.tile([128, 128], bf16, tag=f"dg{t}_{tap}")
                nc.vector.tensor_scalar_mul(d[:], ident[:], dww_sb[t][:, tap:tap + 1])
                diag_sb[(t, tap)] = d
        wpT_sb = [consts.tile([128, 192], bf16, tag=f"wpT{i}", name=f"wpT{i}")
                  for i in range(2)]
        for i in range(2):
            nc.sync.dma_start(out=wpT_sb[i][:], in_=wpT_ext[i])
        pb_sb = consts.tile([1, 192], bf16, tag="pb")
        nc.sync.dma_start(out=pb_sb[:], in_=pb_ext[0:1, :])
        scl_sb = [consts.tile([96, 1], f32, tag=f"scl{j}", name=f"scl{j}")
                  for j in range(2)]
        for j in range(2):
            nc.sync.dma_start(out=scl_sb[j][:], in_=scl_ext[j])

        v_spill = dram.tile([C, S], bf16)
        ar_in = dram.tile([96, 384], f32)
        ar_out = dram.tile([96, 384], f32)
        ones_row = consts.tile([1, 512], bf16, tag="ones")
        nc.vector.memset(ones_row[:], 1.0)

        mult, add = mybir.AluOpType.mult, mybir.AluOpType.add

        def emit_dw_half(t, c, inb, r0, acc, mode):
            """acc[:, 0:8, :] = dw of inb rows r0..r0+10 (out rows r0..r0+8)."""
            dww, dwb = dww_sb[t], dwb_sb[t]
            if mode == "dve":
                nc.vector.tensor_scalar(
                    out=acc[:], in0=inb[:, r0 + 1:r0 + 9, 1:W + 1],
                    scalar1=dww[:, 4:5], scalar2=dwb[:, 0:1], op0=mult, op1=add)
                for tap, (dy, dx) in enumerate(TAPS):
                    if tap == 4:
                        continue
                    sh = inb[:, r0 + dy:r0 + dy + 8, dx:dx + W]
                    tmp = tmpd.tile([128, 8, W], bf16, tag="td")
                    nc.vector.tensor_scalar_mul(tmp[:], sh, dww[:, tap:tap + 1])
                    nc.vector.tensor_tensor(acc[:], acc[:], tmp[:], add)
            elif mode == "actgp":
                nc.scalar.activation(acc[:], inb[:, r0 + 1:r0 + 9, 1:W + 1],
                                     Ident, bias=dwb[:, 0:1], scale=dww[:, 4:5])
                for tap, (dy, dx) in enumerate(TAPS):
                    if tap == 4:
                        continue
                    sh = inb[:, r0 + dy:r0 + dy + 8, dx:dx + W]
                    tmp = tmpg.tile([128, 8, W], bf16, tag="tg")
                    nc.scalar.mul(tmp[:], sh, dww[:, tap:tap + 1])
                    nc.gpsimd.tensor_tensor(acc[:], acc[:], tmp[:], add)
            elif mode == "pe":
                for g in range(4):
                    ps = ps_row.tile([128, 2, W], f32, tag="ps_row")
                    for tap, (dy, dx) in enumerate(TAPS):
                        nc.tensor.matmul(
                            ps[:], diag_sb[(t, tap)][:],
                            inb[:, r0 + 2 * g + dy:r0 + 2 * g + dy + 2, dx:dx + W],
                            start=(tap == 0), stop=(tap == 8))
                    nc.scalar.activation(acc[:, 2 * g:2 * g + 2, :], ps[:],
                                         Ident, bias=dwb[:, 0:1])
            else:
                raise ValueError(mode)

        # ---------------- pass 1 ----------------
        gram_ps = ps_gram.tile([96, 384], f32, tag="gps", name="gram_ps")
        prev_inb = [None] * 5
        prev_accs = None   # (chunk, [acc halves per tile 0..2])
        inb4 = None

        def emit_gram(c, qk_accs):
            """transposes + gram matmuls for chunk c given acc halves of t0..t2."""
            for sb in range(2 * R):
                half, sbh = sb // R, sb % R
                trp = ps_tr.tile([128, 384], bf16, tag="tr")
                for t in range(3):
                    blk = qk_accs[t][half].rearrange("p r w -> p (r w)")[
                        :, 128 * sbh:128 * (sbh + 1)]
                    nc.tensor.transpose(trp[:, 128 * t:128 * (t + 1)], blk, ident[:])
                qkT = qkt.tile([128, 384], bf16, tag="qkT")
                qkT4 = qkT.rearrange("p (pr g cc) -> p pr g cc", pr=4, g=2)
                trq = trp[:, 0:192].rearrange("p (pr cc) -> p pr cc", pr=4)
                trk = trp[:, 192:384].rearrange("p (pr cc) -> p pr cc", pr=4)
                if QKT_SPLIT and sb % 2 == 0:
                    nc.vector.tensor_copy(qkT4[:, :, 0, :], trq)
                    nc.vector.tensor_copy(qkT4[:, :, 1, :], trk)
                else:
                    nc.scalar.copy(qkT4[:, :, 0, :], trq)
                    nc.scalar.copy(qkT4[:, :, 1, :], trk)
                for p in range(4):
                    lhs = qkT[:, 96 * p:96 * (p + 1)]
                    nc.tensor.matmul(gram_ps[:, 96 * p:96 * (p + 1)], lhs, lhs,
                                     start=(c == 0 and sb == 0),
                                     stop=(c == CHUNKS - 1 and sb == 2 * R - 1),
                                     skip_group_check=True)

        for c in range(CHUNKS):
            xa = xpool.tile([128, R + 2, W], bf16, tag="xa")
            xb = xpool.tile([65, R + 2, W], bf16, tag="xb")
            nc.sync.dma_start(out=xa[:], in_=x_ext[0:128, :, c, :])
            nc.sync.dma_start(out=xb[:], in_=x_ext[128:KAUG, :, c, :])

            # ---- qkv 1x1 conv into padded inb buffers ----
            cur_inb = []
            for t in range(4):
                inb = inbp[t].tile([128, R + 2, W + 2], bf16, tag=f"ib{t}")
                if c == 0:
                    nc.vector.memset(inb[:, :, 0:1], 0.0)
                    nc.vector.memset(inb[:, :, W + 1:W + 2], 0.0)
                    g0 = 0
                else:
                    nc.vector.memset(inb[:, 2:R + 2, 0:1], 0.0)
                    nc.vector.memset(inb[:, 2:R + 2, W + 1:W + 2], 0.0)
                    nc.vector.tensor_copy(inb[:, 0:2, :], prev_inb[t][:, R:R + 2, :])
                    g0 = 1
                for g in range(g0, 9):
                    ps = ps_row.tile([128, 2, W], f32, tag="ps_row")
                    nc.tensor.matmul(ps[:], wq_sb[t][0][:],
                                     xa[:, 2 * g:2 * g + 2, :], start=True, stop=False)
                    nc.tensor.matmul(ps[:], wq_sb[t][1][:],
                                     xb[:, 2 * g:2 * g + 2, :], start=False, stop=True)
                    nc.scalar.copy(inb[:, 2 * g:2 * g + 2, 1:W + 1], ps[:])
                cur_inb.append(inb)

            # ---- tile 4: two chunks packed on 128 partitions ----
            if c % 2 == 0:
                inb4 = inbp[4].tile([128, R + 2, W + 2], bf16, tag="ib4")
                nc.vector.memset(inb4[:, :, 0:1], 0.0)
                nc.vector.memset(inb4[:, :, W + 1:W + 2], 0.0)
            hb = (c % 2) * 64
            for g in range(9):
                ps = ps_row.tile([128, 2, W], f32, tag="ps_row")
                nc.tensor.matmul(ps[hb:hb + 64, :, :], wq_sb[4][0][:, hb:hb + 64],
                                 xa[:, 2 * g:2 * g + 2, :], start=True, stop=False)
                nc.tensor.matmul(ps[hb:hb + 64, :, :], wq_sb[4][1][:, hb:hb + 64],
                                 xb[:, 2 * g:2 * g + 2, :], start=False, stop=True)
                nc.scalar.copy(inb4[hb:hb + 64, 2 * g:2 * g + 2, 1:W + 1],
                               ps[hb:hb + 64, :, :])

            # ---- depthwise ----
            accs = {}
            for t in range(4):
                mode = assign[t][c]
                halves = []
                for h in range(2):
                    acc = accp[t].tile([128, 8, W], bf16, tag=f"ac{t}")
                    emit_dw_half(t, c, cur_inb[t], 8 * h, acc, mode)
                    halves.append(acc)
                accs[t] = halves
            if c % 2 == 1:
                for h in range(2):
                    acc = accp[4].tile([128, 8, W], bf16, tag="ac4")
                    emit_dw_half(4, c, inb4, 8 * h, acc, t4_mode)
                    for half_c, p0 in ((c - 1, 0), (c, 64)):
                        nc.sync.dma_start(
                            out=v_spill[128:192,
                                        half_c * R * W + h * 8 * W:
                                        half_c * R * W + (h + 1) * 8 * W],
                            in_=acc[p0:p0 + 64].rearrange("p r w -> p (r w)"))

            # ---- v spill for tile 3 ----
            for h in range(2):
                nc.sync.dma_start(
                    out=v_spill[0:128, c * R * W + h * 8 * W:
                                c * R * W + (h + 1) * 8 * W],
                    in_=accs[3][h].rearrange("p r w -> p (r w)"))

            # ---- transposes + gram for previous chunk ----
            if prev_accs is not None:
                emit_gram(prev_accs[0], prev_accs[1])
            prev_accs = (c, [accs[0], accs[1], accs[2]])
            prev_inb = cur_inb
        emit_gram(prev_accs[0], prev_accs[1])

        # ---------------- stats AllReduce ----------------
        gram_sb = smallp.tile([96, 384], f32, tag="gsb", name="gram_sb")
        nc.scalar.copy(gram_sb[:], gram_ps[:])
        nc.sync.dma_start(out=ar_in[:], in_=gram_sb[:])
        nc.gpsimd.collective_compute(
            "AllReduce", mybir.AluOpType.add,
            replica_groups=[[0, 1], [2, 3], [4, 5], [6, 7]],
            ins=[ar_in.opt()], outs=[ar_out.opt()])

        # ---- extract S / diag(qq) / diag(kk) per head-group j (heads 4j..4j+3)
        GW = 384
        attn_bf = []
        for j in range(2):
            st = smallp.tile([96, 24], f32, tag=f"S{j}")
            qt = smallp.tile([96, 1], f32, tag=f"qq{j}")
            kt = smallp.tile([96, 24], f32, tag=f"kk{j}")
            for l in range(4):
                h = 4 * j + l
                p, d = h // 2, h % 2
                nc.sync.dma_start(
                    out=st[24 * l:24 * l + 24, :],
                    in_=bass.AP(tensor=ar_out.tensor,
                                offset=ar_out.offset + (24 * d) * GW + 96 * p + 48 + 24 * d,
                                ap=[[GW, 24], [1, 24]]))
                nc.sync.dma_start(
                    out=qt[24 * l:24 * l + 24, :],
                    in_=bass.AP(tensor=ar_out.tensor,
                                offset=ar_out.offset + (24 * d) * GW + 96 * p + 24 * d,
                                ap=[[GW + 1, 24], [1, 1]]))
                nc.sync.dma_start(
                    out=kt[24 * l:24 * l + 24, :],
                    in_=bass.AP(tensor=ar_out.tensor,
                                offset=ar_out.offset + (48 + 24 * d) * GW + 96 * p + 48 + 24 * d,
                                ap=[[0, 24], [GW + 1, 24]]))

            iq = smallp.tile([96, 1], f32, tag=f"iq{j}")
            nc.scalar.sqrt(iq[:], qt[:])
            nc.vector.tensor_scalar_max(iq[:], iq[:], EPS)
            nc.vector.reciprocal(iq[:], iq[:])
            nc.vector.tensor_tensor(iq[:], iq[:], scl_sb[j][:], mult)
            ik = smallp.tile([96, 24], f32, tag=f"ik{j}")
            nc.scalar.sqrt(ik[:], kt[:])
            nc.vector.tensor_scalar_max(ik[:], ik[:], EPS)
            nc.vector.reciprocal(ik[:], ik[:])

            nc.vector.tensor_scalar_mul(st[:], st[:], iq[:, 0:1])
            nc.vector.tensor_tensor(st[:], st[:], ik[:], mult)
            rmax = smallp.tile([96, 1], f32, tag=f"rm{j}")
            nc.vector.reduce_max(rmax[:], st[:], axis=mybir.AxisListType.X)
            nc.vector.tensor_scalar(out=st[:], in0=st[:], scalar1=rmax[:, 0:1],
                                    scalar2=None, op0=mybir.AluOpType.subtract)
            nc.scalar.activation(st[:], st[:], mybir.ActivationFunctionType.Exp)
            rsum = smallp.tile([96, 1], f32, tag=f"rs{j}")
            nc.vector.reduce_sum(rsum[:], st[:], axis=mybir.AxisListType.X)
            nc.vector.reciprocal(rsum[:], rsum[:])
            ab = smallp.tile([96, 24], bf16, tag=f"at{j}")
            nc.vector.tensor_scalar_mul(ab[:], st[:], rsum[:, 0:1])
            attn_bf.append(ab)

        # ---- assemble block-diagonal A (un-transposed) ----
        aA = smallp.tile([128, 128], bf16, tag="aA")
        aB = smallp.tile([64, 128], bf16, tag="aB")
        aC = smallp.tile([128, 64], bf16, tag="aC")
        aD = smallp.tile([64, 64], bf16, tag="aD")
        for tl in (aA, aB, aC, aD):
            nc.vector.memset(tl[:], 0.0)
        for h in range(5):
            nc.sync.dma_start(out=aA[24 * h:24 * h + 24, 24 * h:24 * h + 24],
                              in_=attn_bf[h // 4][(h % 4) * 24:(h % 4) * 24 + 24, :])
        nc.sync.dma_start(out=aA[120:128, 120:128], in_=attn_bf[1][24:32, 0:8])
        nc.sync.dma_start(out=aB[0:16, 120:128], in_=attn_bf[1][32:48, 0:8])
        nc.sync.dma_start(out=aC[120:128, 0:16], in_=attn_bf[1][24:32, 8:24])
        nc.sync.dma_start(out=aD[0:16, 0:16], in_=attn_bf[1][32:48, 8:24])
        nc.sync.dma_start(out=aD[16:40, 16:40], in_=attn_bf[1][48:72, :])
        nc.sync.dma_start(out=aD[40:64, 40:64], in_=attn_bf[1][72:96, :])

        # ---- PA = P @ A as pass-2 lhsT:  paT[d, o] ----
        pa0 = ps_row.tile([128, 192], f32, tag="ps_row")
        nc.tensor.matmul(pa0[:], aA[:], wpT_sb[0][:], start=True, stop=False)
        nc.tensor.matmul(pa0[:], aB[:], wpT_sb[1][0:64, :], start=False, stop=True)
        pa1 = ps_row.tile([64, 192], f32, tag="ps_row")
        nc.tensor.matmul(pa1[:], aC[:], wpT_sb[0][:], start=True, stop=False)
        nc.tensor.matmul(pa1[:], aD[:], wpT_sb[1][0:64, :], start=False, stop=True)
        paT0 = smallp.tile([128, 192], bf16, tag="paT0")
        paT1 = smallp.tile([65, 192], bf16, tag="paT1")
        nc.scalar.copy(paT0[:], pa0[:])
        nc.scalar.copy(paT1[0:64, :], pa1[:])
        nc.sync.dma_start(out=paT1[64:65, :], in_=pb_sb[:])

        # ---------------- pass 2: y = PA @ v (+bias via ones row) ----------------
        NU = 512
        for u in range(S // NU):
            sl = slice(u * NU, (u + 1) * NU)
            v0 = iop.tile([128, NU], bf16, tag="v0")
            v1 = iop.tile([65, NU], bf16, tag="v1")
            nc.sync.dma_start(out=v0[:], in_=v_spill[0:128, sl])
            nc.sync.dma_start(out=v1[0:64, :], in_=v_spill[128:192, sl])
            nc.vector.tensor_copy(v1[64:65, :], ones_row[:])
            py0 = ps_row.tile([128, NU], f32, tag="ps_row")
            py1 = ps_row.tile([64, NU], f32, tag="ps_row")
            nc.tensor.matmul(py0[:], paT0[:, 0:128], v0[:], start=True, stop=False)
            nc.tensor.matmul(py0[:], paT1[:, 0:128], v1[:], start=False, stop=True)
            nc.tensor.matmul(py1[:], paT0[:, 128:192], v0[:], start=True, stop=False)
            nc.tensor.matmul(py1[:], paT1[:, 128:192], v1[:], start=False, stop=True)
            y0 = iop.tile([128, NU], bf16, tag="y0")
            y1 = iop.tile([64, NU], bf16, tag="y1")
            nc.scalar.copy(y0[:], py0[:])
            nc.vector.tensor_copy(y1[:], py1[:])
            nc.sync.dma_start(out=y_ext[0:128, sl], in_=y0[:])
            nc.sync.dma_start(out=y_ext[128:192, sl], in_=y1[:])

    nc.compile()
    return nc


def _host_prep(x, qkv_w, qkv_b, dw_w, dw_b, scale, proj_w, proj_b):
    qkv_w = np.asarray(qkv_w)[:, :, 0, 0].astype(np.float32)
    qkv_b = np.asarray(qkv_b).astype(np.float32)
    dw_w = np.asarray(dw_w)[:, 0].astype(np.float32).reshape(C3, 9)
    dw_b = np.asarray(dw_b).astype(np.float32)
    scale = np.asarray(scale)[0, :, 0, 0].astype(np.float32)
    proj_w = np.asarray(proj_w)[:, :, 0, 0].astype(np.float32)
    proj_b = np.asarray(proj_b).astype(np.float32)
    x = np.asarray(x).astype(np.float32)

    wa = np.concatenate([qkv_w, qkv_b[:, None]], axis=1)
    wq = np.zeros((5, KAUG, 128), np.float32)
    dww = np.zeros((5, 128, 9), np.float32)
    dwb = np.zeros((5, 128, 1), np.float32)
    moff = [0, 128, 256, 384, 512, 576]
    for t in range(5):
        msz = moff[t + 1] - moff[t]
        wq[t, :, 0:msz] = wa[moff[t]:moff[t + 1]].T
        dww[t, 0:msz] = dw_w[moff[t]:moff[t + 1]]
        dwb[t, 0:msz, 0] = dw_b[moff[t]:moff[t + 1]]
    # tile 4: duplicate the 64 channels onto partitions 64..127
    wq[4][:, 64:128] = wq[4][:, 0:64]
    dww[4][64:128] = dww[4][0:64]
    dwb[4][64:128] = dwb[4][0:64]

    wpT = np.zeros((2, 128, 192), np.float32)
    wpT[0] = proj_w[:, 0:128].T
    wpT[1, 0:64] = proj_w[:, 128:192].T
    pb = proj_b.reshape(1, 192)
    scl = np.repeat(scale, HC).astype(np.float32).reshape(2, 96, 1)

    shared = {
        "wq": wq.astype(BF16), "dww": dww, "dwb": dwb,
        "wpT": wpT.astype(BF16), "pb": pb.astype(BF16), "scl": scl,
    }
    in_maps = []
    for core in range(N_CORES):
        b, half = core // 2, core % 2
        r0 = half * RH
        xs = np.zeros((KAUG, RH + 2, W), np.float32)
        lo, hi = r0 - 1, r0 + RH + 1
        slo, shi = max(lo, 0), min(hi, H)
        xs[0:C, slo - lo:shi - lo, :] = x[b, :, slo:shi, :]
        xs[C, slo - lo:shi - lo, :] = 1.0
        xc = np.zeros((KAUG, R + 2, CHUNKS, W), np.float32)
        for c in range(CHUNKS):
            xc[:, :, c, :] = xs[:, c * R:c * R + R + 2, :]
        in_maps.append({"x": xc.astype(BF16), **shared})
    return in_maps


def kernel(**inputs):
    if "nc" not in _COMPILED:
        _COMPILED["nc"] = _build_nc()
    nc = _COMPILED["nc"]
    in_maps = _host_prep(**inputs)
    last_err = None
    for _attempt in range(3):
        try:
            res = run_bass_kernel_spmd(nc, in_maps, list(range(N_CORES)))
            break
        except Exception as e:
            last_err = e
    else:
        raise last_err
    y = np.zeros((B, C, H, W), np.float32)
    for core in range(N_CORES):
        b, half = core // 2, core % 2
        y[b, :, half * RH:half * RH + RH, :] = (
            res.results[core]["y"].astype(np.float32).reshape(C, RH, W))
    return y


# revision 24
# speedup vs baseline: 1.1277x; 1.1277x over previous
"""Trainium2 Bass kernel for nn_Attention_34325378629934 (XCA-style channel attention).

Sharding: 8 cores = 4 batches x 2 spatial halves (128 rows each).
Per core, pass 1 (per 16-row chunk):
  1x1 qkv conv as PE matmul (bias via ones-channel, K=193), 16 rows/chunk
    with a 2-row DVE carry of the depthwise halo from the previous chunk
  depthwise 3x3 split across engines per (tile, chunk):
    dve:   tensor_scalar (4x) + tensor_tensor (2x) per tap
    pe:    9 PSUM-accumulated matmuls with diagonal lhsT (shifts via AP offsets)
    actgp: ACT per-partition scale-mul + GpSimd tensor_tensor add
  tile 4 (64 ch) packs two chunks onto 128 partitions (weights duplicated)
  q/k head Gram via PE transpose + PSUM-accumulated PE matmuls
  pairwise AllReduce of Gram stats between the 2 cores of each batch
Pass 2: y = (P @ blockdiag(attn)) @ v with PA computed on device; proj bias via
  an appended ones-row in v; y DMA'd straight from PSUM (fp32).
"""
import sys
from contextlib import ExitStack

sys.path.insert(0, "/opt/trn_rl_repo")

import numpy as np
import ml_dtypes

import concourse.bass as bass
import concourse.mybir as mybir
import concourse.tile as tile
from concourse import bacc
from concourse.bass_utils import run_bass_kernel_spmd
from concourse.masks import make_identity

BF16 = ml_dtypes.bfloat16
f32 = mybir.dt.float32
bf16 = mybir.dt.bfloat16

N_CORES = 8
B, C, H, W = 4, 192, 256, 256
C3 = 3 * C
HEADS, HC = 8, 24
RH = 128                 # rows per core
S = RH * W               # 32768
R = 16                   # rows per chunk
CHUNKS = RH // R         # 8
KAUG = C + 1             # 193
EPS = 1e-12
Ident = mybir.ActivationFunctionType.Identity

TAPS = [(dy, dx) for dy in range(3) for dx in range(3)]  # center = index 4

# dw engine assignment per (tile, chunk); tile 4 handled per pair ("pe")
ASSIGN = {
    0: ["dve"] * 8,
    1: ["dve"] * 7 + ["pe"],
    2: ["pe"] * 8,
    3: ["actgp"] * 7 + ["dve"],
}
T4_MODE = "pe"
QKT_SPLIT = True   # alternate qkT copies between ACT and DVE

_COMPILED = {}


def _build_nc(cfg=None):
    cfg = cfg or {}
    assign = cfg.get("assign", ASSIGN)
    t4_mode = cfg.get("t4", T4_MODE)
    nc = bacc.Bacc()

    x_ext = nc.declare_dram_parameter("x", [KAUG, R + 2, CHUNKS, W], bf16, isOutput=False)
    wq_ext = nc.declare_dram_parameter("wq", [5, KAUG, 128], bf16, isOutput=False)
    dww_ext = nc.declare_dram_parameter("dww", [5, 128, 9], f32, isOutput=False)
    dwb_ext = nc.declare_dram_parameter("dwb", [5, 128, 1], f32, isOutput=False)
    wpT_ext = nc.declare_dram_parameter("wpT", [2, 128, 192], bf16, isOutput=False)
    pb_ext = nc.declare_dram_parameter("pb", [1, 192], bf16, isOutput=False)
    scl_ext = nc.declare_dram_parameter("scl", [2, 96, 1], f32, isOutput=False)
    y_ext = nc.declare_dram_parameter("y", [C, S], bf16, isOutput=True)

    with tile.TileContext(nc) as tc, ExitStack() as ctx:
        consts = ctx.enter_context(tc.tile_pool(name="consts", bufs=1))
        xpool = ctx.enter_context(tc.tile_pool(name="xpool", bufs=2))
        inbp = [ctx.enter_context(tc.tile_pool(name=f"inb{t}", bufs=2))
                for t in range(5)]
        accp = [ctx.enter_context(tc.tile_pool(name=f"acc{t}", bufs=2))
                for t in range(5)]
        tmpd = ctx.enter_context(tc.tile_pool(name="tmpd", bufs=1))
        tmpg = ctx.enter_context(tc.tile_pool(name="tmpg", bufs=2))
        qkt = ctx.enter_context(tc.tile_pool(name="qkt", bufs=2))
        smallp = ctx.enter_context(tc.tile_pool(name="smallp", bufs=1))
        iop = ctx.enter_context(tc.tile_pool(name="iop", bufs=2))
        dram = ctx.enter_context(tc.tile_pool(name="dram", bufs=1, space="DRAM"))
        ps_row = ctx.enter_context(tc.tile_pool(name="ps_row", bufs=2, space="PSUM"))
        ps_tr = ctx.enter_context(tc.tile_pool(name="ps_tr", bufs=2, space="PSUM"))
        ps_gram = ctx.enter_context(tc.tile_pool(name="ps_gram", bufs=1, space="PSUM"))

        # ---------------- constants ----------------
        ident = consts.tile([128, 128], bf16)
        make_identity(nc, ident)
        wq_sb = []
        for t in range(5):
            k0 = consts.tile([128, 128], bf16, tag=f"wq{t}a")
            k1 = consts.tile([65, 128], bf16, tag=f"wq{t}b")
            nc.sync.dma_start(out=k0[:], in_=wq_ext[t, 0:128, :])
            nc.sync.dma_start(out=k1[:], in_=wq_ext[t, 128:KAUG, :])
            wq_sb.append((k0, k1))
        dww_sb, dwb_sb = [], []
        for t in range(5):
            dwt = consts.tile([128, 9], f32, tag=f"dww{t}")
            nc.sync.dma_start(out=dwt[:], in_=dww_ext[t])
            dww_sb.append(dwt)
            dbt = consts.tile([128, 1], f32, tag=f"dwb{t}")
            nc.sync.dma_start(out=dbt[:], in_=dwb_ext[t])
            dwb_sb.append(dbt)
        # diagonal dw-weight matrices for the PE path (only for PE-assigned tiles)
        pe_tiles = {t for t in range(4) if "pe" in assign[t]}
        if t4_mode == "pe":
            pe_tiles.add(4)
        diag_sb = {}
        for t in sorted(pe_tiles):
            for tap in range(9):
                d = consts.tile([128, 128], bf16, tag=f"dg{t}_{tap}")
                nc.vector.tensor_scalar_mul(d[:], ident[:], dww_sb[t][:, tap:tap + 1])
                diag_sb[(t, tap)] = d
        wpT_sb = [consts.tile([128, 192], bf16, tag=f"wpT{i}", name=f"wpT{i}")
                  for i in range(2)]
        for i in range(2):
            nc.sync.dma_start(out=wpT_sb[i][:], in_=wpT_ext[i])
        pb_sb = consts.tile([1, 192], bf16, tag="pb")
        nc.sync.dma_start(out=pb_sb[:], in_=pb_ext[0:1, :])
        scl_sb = [consts.tile([96, 1], f32, tag=f"scl{j}", name=f"scl{j}")
                  for j in range(2)]
        for j in range(2):
            nc.sync.dma_start(out=scl_sb[j][:], in_=scl_ext[j])

        v_spill = dram.tile([C, S], bf16)
        ar_in = dram.tile([96, 384], f32)
        ar_out = dram.tile([96, 384], f32)
        ones_row = consts.tile([1, 512], bf16, tag="ones")
        nc.vector.memset(ones_row[:], 1.0)

        mult, add = mybir.AluOpType.mult, mybir.AluOpType.add

        def emit_dw_half(t, c, inb, r0, acc, mode):
            """acc[:, 0:8, :] = dw of inb rows r0..r0+10 (out rows r0..r0+8)."""
            dww, dwb = dww_sb[t], dwb_sb[t]
            if mode == "dve":
                nc.vector.tensor_scalar(
                    out=acc[:], in0=inb[:, r0 + 1:r0 + 9, 1:W + 1],
                    scalar1=dww[:, 4:5], scalar2=dwb[:, 0:1], op0=mult, op1=add)
                for tap, (dy, dx) in enumerate(TAPS):
                    if tap == 4:
                        continue
                    sh = inb[:, r0 + dy:r0 + dy + 8, dx:dx + W]
                    tmp = tmpd.tile([128, 8, W], bf16, tag="td")
                    nc.vector.tensor_scalar_mul(tmp[:], sh, dww[:, tap:tap + 1])
                    nc.vector.tensor_tensor(acc[:], acc[:], tmp[:], add)
            elif mode == "actgp":
                nc.scalar.activation(acc[:], inb[:, r0 + 1:r0 + 9, 1:W + 1],
                                     Ident, bias=dwb[:, 0:1], scale=dww[:, 4:5])
                for tap, (dy, dx) in enumerate(TAPS):
                    if tap == 4:
                        continue
                    sh = inb[:, r0 + dy:r0 + dy + 8, dx:dx + W]
                    tmp = tmpg.tile([128, 8, W], bf16, tag="tg")
                    nc.scalar.mul(tmp[:], sh, dww[:, tap:tap + 1])
                    nc.gpsimd.tensor_tensor(acc[:], acc[:], tmp[:], add)
            elif mode == "pe":
                for q in range(2):
                    ps = ps_row.tile([128, 4, W], f32, tag="ps_row")
                    for sg in range(2):
                        g = 2 * q + sg
                        for tap, (dy, dx) in enumerate(TAPS):
                            nc.tensor.matmul(
                                ps[:, 2 * sg:2 * sg + 2, :], diag_sb[(t, tap)][:],
                                inb[:, r0 + 2 * g + dy:r0 + 2 * g + dy + 2, dx:dx + W],
                                start=(tap == 0), stop=(tap == 8))
                    nc.scalar.activation(acc[:, 4 * q:4 * q + 4, :], ps[:],
                                         Ident, bias=dwb[:, 0:1])
            else:
                raise ValueError(mode)

        # ---------------- pass 1 ----------------
        gram_ps = ps_gram.tile([96, 384], f32, tag="gps", name="gram_ps")
        prev_inb = [None] * 5
        prev_accs = None   # (chunk, [acc halves per tile 0..2])
        inb4 = None

        def emit_gram(c, qk_accs):
            """transposes + gram matmuls for chunk c given acc halves of t0..t2.

            Software-pipelined one block deep so PE's gram matmul for block b
            runs while the qkT copy for block b+1 is in flight."""
            pend = None
            for sb in range(2 * R):
                half, sbh = sb // R, sb % R
                trp = ps_tr.tile([128, 384], bf16, tag="tr")
                for t in range(3):
                    blk = qk_accs[t][half].rearrange("p r w -> p (r w)")[
                        :, 128 * sbh:128 * (sbh + 1)]
                    nc.tensor.transpose(trp[:, 128 * t:128 * (t + 1)], blk, ident[:])
                qkT = qkt.tile([128, 384], bf16, tag="qkT")
                qkT4 = qkT.rearrange("p (pr g cc) -> p pr g cc", pr=4, g=2)
                trq = trp[:, 0:192].rearrange("p (pr cc) -> p pr cc", pr=4)
                trk = trp[:, 192:384].rearrange("p (pr cc) -> p pr cc", pr=4)
                if QKT_SPLIT and sb % 2 == 0:
                    nc.vector.tensor_copy(qkT4[:, :, 0, :], trq)
                    nc.vector.tensor_copy(qkT4[:, :, 1, :], trk)
                else:
                    nc.scalar.copy(qkT4[:, :, 0, :], trq)
                    nc.scalar.copy(qkT4[:, :, 1, :], trk)
                if pend is not None:
                    _emit_gram_mm(*pend)
                pend = (c, sb, qkT)
            _emit_gram_mm(*pend)

        def _emit_gram_mm(c, sb, qkT):
            for p in range(4):
                lhs = qkT[:, 96 * p:96 * (p + 1)]
                nc.tensor.matmul(gram_ps[:, 96 * p:96 * (p + 1)], lhs, lhs,
                                 start=(c == 0 and sb == 0),
                                 stop=(c == CHUNKS - 1 and sb == 2 * R - 1),
                                 skip_group_check=True)

        for c in range(CHUNKS):
            xa = xpool.tile([128, R + 2, W], bf16, tag="xa")
            xb = xpool.tile([65, R + 2, W], bf16, tag="xb")
            nc.sync.dma_start(out=xa[:], in_=x_ext[0:128, :, c, :])
            nc.sync.dma_start(out=xb[:], in_=x_ext[128:KAUG, :, c, :])

            # ---- transposes + gram for previous chunk (PE starts here) ----
            if prev_accs is not None:
                emit_gram(prev_accs[0], prev_accs[1])

            # ---- qkv 1x1 conv into padded inb buffers (4-row PSUM tiles) ----
            cur_inb = []
            for t in range(4):
                inb = inbp[t].tile([128, R + 2, W + 2], bf16, tag=f"ib{t}")
                if c == 0:
                    nc.vector.memset(inb[:, :, 0:1], 0.0)
                    nc.vector.memset(inb[:, :, W + 1:W + 2], 0.0)
                    g0 = 0
                else:
                    nc.vector.memset(inb[:, 2:R + 2, 0:1], 0.0)
                    nc.vector.memset(inb[:, 2:R + 2, W + 1:W + 2], 0.0)
                    nc.vector.tensor_copy(inb[:, 0:2, :], prev_inb[t][:, R:R + 2, :])
                    g0 = 1
                gi = g0
                while gi < 9:
                    take = min(2, 9 - gi)
                    ps = ps_row.tile([128, 4, W], f32, tag="ps_row")
                    for s in range(take):
                        g = gi + s
                        nc.tensor.matmul(ps[:, 2 * s:2 * s + 2, :], wq_sb[t][0][:],
                                         xa[:, 2 * g:2 * g + 2, :],
                                         start=True, stop=False)
                        nc.tensor.matmul(ps[:, 2 * s:2 * s + 2, :], wq_sb[t][1][:],
                                         xb[:, 2 * g:2 * g + 2, :],
                                         start=False, stop=True)
                    nc.scalar.copy(inb[:, 2 * gi:2 * gi + 2 * take, 1:W + 1],
                                   ps[:, 0:2 * take, :])
                    gi += take
                cur_inb.append(inb)

            # ---- tile 4: two chunks packed on 128 partitions ----
            if c % 2 == 0:
                inb4 = inbp[4].tile([128, R + 2, W + 2], bf16, tag="ib4")
                nc.vector.memset(inb4[:, :, 0:1], 0.0)
                nc.vector.memset(inb4[:, :, W + 1:W + 2], 0.0)
            hb = (c % 2) * 64
            gi = 0
            while gi < 9:
                take = min(2, 9 - gi)
                ps = ps_row.tile([128, 4, W], f32, tag="ps_row")
                for s in range(take):
                    g = gi + s
                    nc.tensor.matmul(ps[hb:hb + 64, 2 * s:2 * s + 2, :],
                                     wq_sb[4][0][:, hb:hb + 64],
                                     xa[:, 2 * g:2 * g + 2, :], start=True, stop=False)
                    nc.tensor.matmul(ps[hb:hb + 64, 2 * s:2 * s + 2, :],
                                     wq_sb[4][1][:, hb:hb + 64],
                                     xb[:, 2 * g:2 * g + 2, :], start=False, stop=True)
                nc.scalar.copy(inb4[hb:hb + 64, 2 * gi:2 * gi + 2 * take, 1:W + 1],
                               ps[hb:hb + 64, 0:2 * take, :])
                gi += take

            # ---- depthwise ----
            accs = {}
            for t in range(4):
                mode = assign[t][c]
                halves = []
                for h in range(2):
                    acc = accp[t].tile([128, 8, W], bf16, tag=f"ac{t}")
                    emit_dw_half(t, c, cur_inb[t], 8 * h, acc, mode)
                    halves.append(acc)
                accs[t] = halves
            if c % 2 == 1:
                for h in range(2):
                    acc = accp[4].tile([128, 8, W], bf16, tag="ac4")
                    emit_dw_half(4, c, inb4, 8 * h, acc, t4_mode)
                    for half_c, p0 in ((c - 1, 0), (c, 64)):
                        nc.sync.dma_start(
                            out=v_spill[128:192,
                                        half_c * R * W + h * 8 * W:
                                        half_c * R * W + (h + 1) * 8 * W],
                            in_=acc[p0:p0 + 64].rearrange("p r w -> p (r w)"))

            # ---- v spill for tile 3 ----
            for h in range(2):
                nc.sync.dma_start(
                    out=v_spill[0:128, c * R * W + h * 8 * W:
                                c * R * W + (h + 1) * 8 * W],
                    in_=accs[3][h].rearrange("p r w -> p (r w)"))

            prev_accs = (c, [accs[0], accs[1], accs[2]])
            prev_inb = cur_inb
        emit_gram(prev_accs[0], prev_accs[1])

        # ---------------- stats AllReduce ----------------
        gram_sb = smallp.tile([96, 384], f32, tag="gsb", name="gram_sb")
        nc.scalar.copy(gram_sb[:], gram_ps[:])
        nc.sync.dma_start(out=ar_in[:], in_=gram_sb[:])
        nc.gpsimd.collective_compute(
            "AllReduce", mybir.AluOpType.add,
            replica_groups=[[0, 1], [2, 3], [4, 5], [6, 7]],
            ins=[ar_in.opt()], outs=[ar_out.opt()])

        # ---- extract S / diag(qq) / diag(kk) per head-group j (heads 4j..4j+3)
        GW = 384
        attn_bf = []
        for j in range(2):
            st = smallp.tile([96, 24], f32, tag=f"S{j}")
            qt = smallp.tile([96, 1], f32, tag=f"qq{j}")
            kt = smallp.tile([96, 24], f32, tag=f"kk{j}")
            for l in range(4):
                h = 4 * j + l
                p, d = h // 2, h % 2
                nc.sync.dma_start(
                    out=st[24 * l:24 * l + 24, :],
                    in_=bass.AP(tensor=ar_out.tensor,
                                offset=ar_out.offset + (24 * d) * GW + 96 * p + 48 + 24 * d,
                                ap=[[GW, 24], [1, 24]]))
                nc.sync.dma_start(
                    out=qt[24 * l:24 * l + 24, :],
                    in_=bass.AP(tensor=ar_out.tensor,
                                offset=ar_out.offset + (24 * d) * GW + 96 * p + 24 * d,
                                ap=[[GW + 1, 24], [1, 1]]))
                nc.sync.dma_start(
                    out=kt[24 * l:24 * l + 24, :],
                    in_=bass.AP(tensor=ar_out.tensor,
                                offset=ar_out.offset + (48 + 24 * d) * GW + 96 * p + 48 + 24 * d,
                                ap=[[0, 24], [GW + 1, 24]]))

            iq = smallp.tile([96, 1], f32, tag=f"iq{j}")
            nc.scalar.sqrt(iq[:], qt[:])
            nc.vector.tensor_scalar_max(iq[:], iq[:], EPS)
            nc.vector.reciprocal(iq[:], iq[:])
            nc.vector.tensor_tensor(iq[:], iq[:], scl_sb[j][:], mult)
            ik = smallp.tile([96, 24], f32, tag=f"ik{j}")
            nc.scalar.sqrt(ik[:], kt[:])
            nc.vector.tensor_scalar_max(ik[:], ik[:], EPS)
            nc.vector.reciprocal(ik[:], ik[:])

            nc.vector.tensor_scalar_mul(st[:], st[:], iq[:, 0:1])
            nc.vector.tensor_tensor(st[:], st[:], ik[:], mult)
            rmax = smallp.tile([96, 1], f32, tag=f"rm{j}")
            nc.vector.reduce_max(rmax[:], st[:], axis=mybir.AxisListType.X)
            nc.vector.tensor_scalar(out=st[:], in0=st[:], scalar1=rmax[:, 0:1],
                                    scalar2=None, op0=mybir.AluOpType.subtract)
            nc.scalar.activation(st[:], st[:], mybir.ActivationFunctionType.Exp)
            rsum = smallp.tile([96, 1], f32, tag=f"rs{j}")
            nc.vector.reduce_sum(rsum[:], st[:], axis=mybir.AxisListType.X)
            nc.vector.reciprocal(rsum[:], rsum[:])
            ab = smallp.tile([96, 24], bf16, tag=f"at{j}")
            nc.vector.tensor_scalar_mul(ab[:], st[:], rsum[:, 0:1])
            attn_bf.append(ab)

        # ---- assemble block-diagonal A (un-transposed) ----
        aA = smallp.tile([128, 128], bf16, tag="aA")
        aB = smallp.tile([64, 128], bf16, tag="aB")
        aC = smallp.tile([128, 64], bf16, tag="aC")
        aD = smallp.tile([64, 64], bf16, tag="aD")
        for tl in (aA, aB, aC, aD):
            nc.vector.memset(tl[:], 0.0)
        for h in range(5):
            nc.sync.dma_start(out=aA[24 * h:24 * h + 24, 24 * h:24 * h + 24],
                              in_=attn_bf[h // 4][(h % 4) * 24:(h % 4) * 24 + 24, :])
        nc.sync.dma_start(out=aA[120:128, 120:128], in_=attn_bf[1][24:32, 0:8])
        nc.sync.dma_start(out=aB[0:16, 120:128], in_=attn_bf[1][32:48, 0:8])
        nc.sync.dma_start(out=aC[120:128, 0:16], in_=attn_bf[1][24:32, 8:24])
        nc.sync.dma_start(out=aD[0:16, 0:16], in_=attn_bf[1][32:48, 8:24])
        nc.sync.dma_start(out=aD[16:40, 16:40], in_=attn_bf[1][48:72, :])
        nc.sync.dma_start(out=aD[40:64, 40:64], in_=attn_bf[1][72:96, :])

        # ---- PA = P @ A as pass-2 lhsT:  paT[d, o] ----
        pa0 = ps_row.tile([128, 192], f32, tag="ps_row")
        nc.tensor.matmul(pa0[:], aA[:], wpT_sb[0][:], start=True, stop=False)
        nc.tensor.matmul(pa0[:], aB[:], wpT_sb[1][0:64, :], start=False, stop=True)
        pa1 = ps_row.tile([64, 192], f32, tag="ps_row")
        nc.tensor.matmul(pa1[:], aC[:], wpT_sb[0][:], start=True, stop=False)
        nc.tensor.matmul(pa1[:], aD[:], wpT_sb[1][0:64, :], start=False, stop=True)
        paT0 = smallp.tile([128, 192], bf16, tag="paT0")
        paT1 = smallp.tile([65, 192], bf16, tag="paT1")
        nc.scalar.copy(paT0[:], pa0[:])
        nc.scalar.copy(paT1[0:64, :], pa1[:])
        nc.sync.dma_start(out=paT1[64:65, :], in_=pb_sb[:])

        # ---------------- pass 2: y = PA @ v (+bias via ones row) ----------------
        NU = 512
        for u in range(S // NU):
            sl = slice(u * NU, (u + 1) * NU)
            v0 = iop.tile([128, NU], bf16, tag="v0")
            v1 = iop.tile([65, NU], bf16, tag="v1")
            nc.sync.dma_start(out=v0[:], in_=v_spill[0:128, sl])
            nc.sync.dma_start(out=v1[0:64, :], in_=v_spill[128:192, sl])
            nc.vector.tensor_copy(v1[64:65, :], ones_row[:])
            py0 = ps_row.tile([128, NU], f32, tag="ps_row")
            py1 = ps_row.tile([64, NU], f32, tag="ps_row")
            nc.tensor.matmul(py0[:], paT0[:, 0:128], v0[:], start=True, stop=False)
            nc.tensor.matmul(py0[:], paT1[:, 0:128], v1[:], start=False, stop=True)
            nc.tensor.matmul(py1[:], paT0[:, 128:192], v0[:], start=True, stop=False)
            nc.tensor.matmul(py1[:], paT1[:, 128:192], v1[:], start=False, stop=True)
            y0 = iop.tile([128, NU], bf16, tag="y0")
            y1 = iop.tile([64, NU], bf16, tag="y1")
            nc.scalar.copy(y0[:], py0[:])
            nc.vector.tensor_copy(y1[:], py1[:])
            nc.sync.dma_start(out=y_ext[0:128, sl], in_=y0[:])
            nc.sync.dma_start(out=y_ext[128:192, sl], in_=y1[:])

    nc.compile()
    return nc


def _host_prep(x, qkv_w, qkv_b, dw_w, dw_b, scale, proj_w, proj_b):
    qkv_w = np.asarray(qkv_w)[:, :, 0, 0].astype(np.float32)
    qkv_b = np.asarray(qkv_b).astype(np.float32)
    dw_w = np.asarray(dw_w)[:, 0].astype(np.float32).reshape(C3, 9)
    dw_b = np.asarray(dw_b).astype(np.float32)
    scale = np.asarray(scale)[0, :, 0, 0].astype(np.float32)
    proj_w = np.asarray(proj_w)[:, :, 0, 0].astype(np.float32)
    proj_b = np.asarray(proj_b).astype(np.float32)
    x = np.asarray(x).astype(np.float32)

    wa = np.concatenate([qkv_w, qkv_b[:, None]], axis=1)
    wq = np.zeros((5, KAUG, 128), np.float32)
    dww = np.zeros((5, 128, 9), np.float32)
    dwb = np.zeros((5, 128, 1), np.float32)
    moff = [0, 128, 256, 384, 512, 576]
    for t in range(5):
        msz = moff[t + 1] - moff[t]
        wq[t, :, 0:msz] = wa[moff[t]:moff[t + 1]].T
        dww[t, 0:msz] = dw_w[moff[t]:moff[t + 1]]
        dwb[t, 0:msz, 0] = dw_b[moff[t]:moff[t + 1]]
    # tile 4: duplicate the 64 channels onto partitions 64..127
    wq[4][:, 64:128] = wq[4][:, 0:64]
    dww[4][64:128] = dww[4][0:64]
    dwb[4][64:128] = dwb[4][0:64]

    wpT = np.zeros((2, 128, 192), np.float32)
    wpT[0] = proj_w[:, 0:128].T
    wpT[1, 0:64] = proj_w[:, 128:192].T
    pb = proj_b.reshape(1, 192)
    scl = np.repeat(scale, HC).astype(np.float32).reshape(2, 96, 1)

    shared = {
        "wq": wq.astype(BF16), "dww": dww, "dwb": dwb,
        "wpT": wpT.astype(BF16), "pb": pb.astype(BF16), "scl": scl,
    }
    in_maps = []
    for core in range(N_CORES):
        b, half = core // 2, core % 2
        r0 = half * RH
        xs = np.zeros((KAUG, RH + 2, W), np.float32)
        lo, hi = r0 - 1, r0 + RH + 1
        slo, shi = max(lo, 0), min(hi, H)
        xs[0:C, slo - lo:shi - lo, :] = x[b, :, slo:shi, :]
        xs[C, slo - lo:shi - lo, :] = 1.0
        xc = np.zeros((KAUG, R + 2, CHUNKS, W), np.float32)
        for c in range(CHUNKS):
            xc[:, :, c, :] = xs[:, c * R:c * R + R + 2, :]
        in_maps.append({"x": xc.astype(BF16), **shared})
    return in_maps


def kernel(**inputs):
    if "nc" not in _COMPILED:
        _COMPILED["nc"] = _build_nc()
    nc = _COMPILED["nc"]
    in_maps = _host_prep(**inputs)
    last_err = None
    for _attempt in range(3):
        try:
            res = run_bass_kernel_spmd(nc, in_maps, list(range(N_CORES)))
            break
        except Exception as e:
            last_err = e
    else:
        raise last_err
    y = np.zeros((B, C, H, W), np.float32)
    for core in range(N_CORES):
        b, half = core // 2, core % 2
        y[b, :, half * RH:half * RH + RH, :] = (
            res.results[core]["y"].astype(np.float32).reshape(C, RH, W))
    return y
